# revision 10
# baseline (speedup 1.0000x reference)
"""NetVLAD pooling kernel for Trainium2 (8 NeuronCores, batch-sharded).

Reference computation (B=32, N=2048, D=512, K=64):
    L = x.reshape(B*N, D) @ clusters                         # [B*N, K]
    A = softmax(BN_train(L), axis=1)                         # batch stats over ALL B*N rows
    a_sum[b] = sum_n A[b,n,:]
    vlad[b]  = einsum('nk,nd->dk', A[b], x[b]) - a_sum[b]*clusters2[0]
    vlad     = intra_normalize_over_D -> flatten -> L2 normalize (== /8)

Device strategy (per core, 4 batches = 8192 rows, x shard 16 MB resident in SBUF):
  Phase 1: PE-transpose x tiles -> xT; L^T[k, n] = clusters^T x^T via f32r matmuls;
           bn_stats/bn_aggr accumulate per-k mean/var over local rows.
  AllReduce [64, 2] of (sum, sumsq) across the 8 cores -> global BN scale/shift
           as per-partition columns [64, 1].
  Phase 2: E^T = exp(scale*L^T + shift) in one ACT op; PE-transpose E^T -> E[n, K];
           row-softmax-normalize into A (f32r); vladT[b][k, d] accumulated by PE
           (lhsT=A, rhs=x); a_sum by PE with ones rhs.
  Epilogue: vladT -= a_sum * clusters2^T; per-k L2 norm over d; * 0.125 (the final
           whole-vector norm is exactly 8 since all 64 rows are unit); PE-transpose
           to [d, k] and DMA out [4, 512, 64].
Host: shard x over batch, run SPMD on cores 0-7, concat + reshape to [32, 32768].
"""

import os
import sys

sys.path.insert(0, "/opt/trn_rl_repo")

STAGE = float(os.environ.get("KERNEL_STAGE", "3"))

import numpy as np

import concourse.bacc as bacc
import concourse.tile as tile
from concourse import mybir
from concourse.bass_utils import run_bass_kernel_spmd
from concourse.masks import make_identity

N_CORES = 8
B, N, D, K = 32, 2048, 512, 64
BL = B // N_CORES            # batches per core
R_LOCAL = BL * N             # rows per core
R_TOTAL = B * N              # rows overall
NBLK = R_LOCAL // 512        # 512-row blocks per core (16)
BN_EPS = 1e-5
NORM_EPS = 1e-12

F32 = mybir.dt.float32
F32R = mybir.dt.float32r
EXPF = mybir.ActivationFunctionType.Exp
SQRTF = mybir.ActivationFunctionType.Sqrt

# which engine evacuates each xT d-chunk from PSUM ("v"=DVE, "s"=ACT)
XT_COPY_ENG = ("v", "s", "v", "s")
LT_COPY_ENG = "s"


def _copy(nc, eng, out, in_):
    if eng == "v":
        nc.vector.tensor_copy(out, in_)
    else:
        nc.scalar.copy(out, in_)


def build():
    nc = bacc.Bacc("TRN2", target_bir_lowering=False, debug=False,
                   num_devices=N_CORES)

    x = nc.dram_tensor("x", [BL, N, D], F32R, kind="ExternalInput")
    cl = nc.dram_tensor("clusters", [D, K], F32R, kind="ExternalInput")
    c2t = nc.dram_tensor("c2t", [K, D], F32, kind="ExternalInput")
    gamma = nc.dram_tensor("gamma", [K, 1], F32, kind="ExternalInput")
    beta = nc.dram_tensor("beta", [K, 1], F32, kind="ExternalInput")
    out = nc.dram_tensor("vlad", [BL, D, K], F32, kind="ExternalOutput")

    with tile.TileContext(nc) as tc:
        with (
            tc.tile_pool(name="const", bufs=1) as const,
            tc.tile_pool(name="xres", bufs=NBLK) as xres,
            tc.tile_pool(name="ltres", bufs=1) as ltres,
            tc.tile_pool(name="xt", bufs=2) as xtp,
            tc.tile_pool(name="et", bufs=2) as etp,
            tc.tile_pool(name="ap", bufs=3) as apool,
            tc.tile_pool(name="ep", bufs=2) as epi,
            tc.tile_pool(name="sm", bufs=4) as sm,
            tc.tile_pool(name="ps_big", bufs=3, space="PSUM") as ps_big,
            tc.tile_pool(name="ps_l", bufs=2, space="PSUM") as ps_l,
            tc.tile_pool(name="ps_a", bufs=2, space="PSUM") as ps_a,
            tc.tile_pool(name="dram", bufs=1, space="DRAM") as dram,
        ):
            # ---- constants ----
            ident = const.tile([128, 128], F32)
            make_identity(nc, ident)
            ident_r = const.tile([128, 128], F32R)
            nc.vector.tensor_copy(ident_r[:], ident[:])
            ident64 = ident[0:64, 0:64]

            cl_sb = const.tile([128, 4, K], F32R)
            nc.sync.dma_start(out=cl_sb, in_=cl[:, :].rearrange("(c p) k -> p c k", p=128))
            c2t_sb = const.tile([K, D], F32)
            nc.sync.dma_start(out=c2t_sb, in_=c2t[:, :])
            gamma_sb = const.tile([K, 1], F32)
            nc.sync.dma_start(out=gamma_sb, in_=gamma[:, :])
            beta_sb = const.tile([K, 1], F32)
            nc.sync.dma_start(out=beta_sb, in_=beta[:, :])
            ones_f = const.tile([128, 1], F32)
            nc.vector.memset(ones_f, 1.0)
            eps_sb = const.tile([K, 1], F32)
            nc.vector.memset(eps_sb, BN_EPS)

            # ---- resident tensors ----
            xs = []
            for t in range(NBLK):
                xt_tile = xres.tile([128, 4, D], F32R, tag="x")
                b_idx, n0 = t // 4, (t % 4) * 512
                nc.sync.dma_start(
                    out=xt_tile,
                    in_=x[b_idx, n0:n0 + 512, :].rearrange("(s p) d -> p s d", p=128),
                )
                xs.append(xt_tile)
            lt = ltres.tile([K, NBLK, 512], F32)         # L^T resident
            stats6 = const.tile([K, NBLK, 6], F32)

            # ---- phase 1: logits + stats ----
            for t in range(NBLK):
                xt = xtp.tile([128, 4, 512], F32R, tag="xt")
                for c in range(4):
                    psx = ps_big.tile([128, 512], F32, tag="psbig")
                    for s in range(4):
                        nc.tensor.transpose(
                            psx[:, s * 128:(s + 1) * 128].bitcast(F32R),
                            xs[t][:, s, c * 128:(c + 1) * 128],
                            ident_r[:],
                        )
                    _copy(nc, XT_COPY_ENG[c], xt[:, c, :], psx[:])
                psl = ps_l.tile([K, 512], F32, tag="psl")
                for c in range(4):
                    nc.tensor.matmul(
                        psl[:], cl_sb[:, c, :], xt[:, c, :],
                        start=(c == 0), stop=(c == 3),
                    )
                nc.vector.bn_stats(out=stats6[:, t, :], in_=psl[:])
                _copy(nc, LT_COPY_ENG, lt[:, t, :], psl[:])

            # ---- global BN stats via AllReduce ----
            mv = sm.tile([K, 2], F32, tag="mv")
            nc.vector.bn_aggr(out=mv[:], in_=stats6[:])
            sums = sm.tile([K, 2], F32, tag="sums")
            # sums[:,0] = mean * R_LOCAL ; sums[:,1] = (var + mean^2) * R_LOCAL
            msq = sm.tile([K, 1], F32, tag="msq")
            nc.vector.tensor_mul(msq[:], mv[:, 0:1], mv[:, 0:1])
            nc.vector.tensor_add(msq[:], msq[:], mv[:, 1:2])
            nc.vector.tensor_scalar_mul(sums[:, 0:1], mv[:, 0:1], float(R_LOCAL))
            nc.vector.tensor_scalar_mul(sums[:, 1:2], msq[:], float(R_LOCAL))

            if STAGE >= 1.5:
                pass
            cc_in = dram.tile([K, 2], F32)
            cc_out = dram.tile([K, 2], F32)
            nc.sync.dma_start(out=cc_in[:], in_=sums[:])
            if STAGE >= 2:
              nc.gpsimd.collective_compute(
                "AllReduce", mybir.AluOpType.add,
                replica_groups=[list(range(N_CORES))],
                ins=[cc_in.opt()], outs=[cc_out.opt()],
              )
            gsum = sm.tile([K, 2], F32, tag="gsum")
            nc.sync.dma_start(out=gsum[:], in_=cc_out[:] if STAGE >= 2 else cc_in[:])

            scale_c = sm.tile([K, 1], F32, tag="scale")
            shift_c = sm.tile([K, 1], F32, tag="shift")
            mean_c = sm.tile([K, 1], F32, tag="mean")
            var_c = sm.tile([K, 1], F32, tag="var")
            nc.vector.tensor_scalar_mul(mean_c[:], gsum[:, 0:1], 1.0 / R_TOTAL)
            nc.vector.tensor_scalar_mul(var_c[:], gsum[:, 1:2], 1.0 / R_TOTAL)
            t0 = sm.tile([K, 1], F32, tag="t0")
            nc.vector.tensor_mul(t0[:], mean_c[:], mean_c[:])
            nc.vector.tensor_sub(var_c[:], var_c[:], t0[:])    # var = E[x^2]-mean^2
            nc.scalar.activation(out=var_c[:], in_=var_c[:], func=SQRTF, bias=eps_sb[:])
            nc.vector.reciprocal(var_c[:], var_c[:])           # rstd
            nc.vector.tensor_mul(scale_c[:], var_c[:], gamma_sb[:])
            nc.vector.tensor_mul(t0[:], mean_c[:], scale_c[:])
            nc.vector.tensor_sub(shift_c[:], beta_sb[:], t0[:])

            # ---- phase 2: softmax + vlad ----
            for b_idx in range(BL if STAGE >= 2.2 else 0):
                psv = ps_l.tile([K, 512], F32, tag="psl")
                psa = ps_a.tile([K, 1], F32, tag="psa")
                for tl in range(4):
                    t = b_idx * 4 + tl
                    et = etp.tile([K, 512], F32, tag="et")
                    nc.scalar.activation(
                        out=et[:], in_=lt[:, t, :], func=EXPF,
                        bias=shift_c[:], scale=scale_c[:],
                    )
                    pse = ps_big.tile([128, 256], F32, tag="psbig")
                    for s in range(4):
                        nc.tensor.transpose(
                            pse[:, s * 64:(s + 1) * 64],
                            et[:, s * 128:(s + 1) * 128],
                            ident64,
                        )
                    rs = sm.tile([128, 4], F32, tag="rs")
                    nc.vector.reduce_sum(
                        out=rs[:], in_=pse[:].rearrange("p (s k) -> p s k", k=64),
                        axis=mybir.AxisListType.X,
                    )
                    rc = sm.tile([128, 4], F32, tag="rc")
                    nc.vector.reciprocal(rc[:], rs[:])
                    a_t = apool.tile([128, 4, K], F32R, tag="a")
                    for s in range(4):
                        nc.vector.tensor_scalar_mul(
                            a_t[:, s, :], pse[:, s * 64:(s + 1) * 64], rc[:, s:s + 1]
                        )
                    if STAGE >= 2.5:
                        for s in range(4):
                            nc.tensor.matmul(
                                psv[:], a_t[:, s, :], xs[t][:, s, :],
                                start=(tl == 0 and s == 0), stop=(tl == 3 and s == 3),
                            )
                        for s in range(4):
                            nc.tensor.matmul(
                                psa[:], a_t[:, s, :].bitcast(F32), ones_f[:],
                                start=(tl == 0 and s == 0), stop=(tl == 3 and s == 3),
                            )

                # epilogue for batch b
                if STAGE < 2.8:
                    continue
                asum = epi.tile([K, 1], F32, tag="asum")
                nc.vector.tensor_copy(asum[:], psa[:])
                if STAGE < 2.81:
                    continue
                tmp = epi.tile([K, D], F32, tag="tmp")
                nc.vector.tensor_scalar_mul(tmp[:], c2t_sb[:], asum[:])
                if STAGE < 2.82:
                    continue
                vl = epi.tile([K, D], F32, tag="vl")
                nc.vector.tensor_sub(vl[:], psv[:], tmp[:])
                if STAGE < 2.83:
                    continue
                sq = epi.tile([K, D], F32, tag="sq")
                nrm = epi.tile([K, 1], F32, tag="nrm")
                nc.vector.tensor_mul(sq[:], vl[:], vl[:])
                nc.vector.reduce_sum(out=nrm[:], in_=sq[:], axis=mybir.AxisListType.X)
                if STAGE < 2.84:
                    continue
                nc.scalar.activation(out=nrm[:], in_=nrm[:], func=SQRTF)
                nc.vector.tensor_scalar_max(nrm[:], nrm[:], NORM_EPS)
                nc.vector.reciprocal(nrm[:], nrm[:])
                nc.vector.tensor_scalar_mul(nrm[:], nrm[:], 0.125)
                if STAGE < 2.85:
                    continue
                vn = epi.tile([K, D], F32, tag="vn")
                nc.vector.tensor_scalar_mul(vn[:], vl[:], nrm[:])

                if STAGE < 2.9:
                    continue
                pso = ps_big.tile([128, 256], F32, tag="psbig")
                for c in range(4):
                    nc.tensor.transpose(
                        pso[:, c * 64:(c + 1) * 64],
                        vn[:, c * 128:(c + 1) * 128],
                        ident64,
                    )
                if STAGE < 2.95:
                    continue
                osb = epi.tile([128, 4, K], F32, tag="osb")
                nc.scalar.copy(osb[:], pso[:].rearrange("p (c k) -> p c k", k=64))
                nc.sync.dma_start(
                    out=out[b_idx].rearrange("(c p) k -> p c k", p=128),
                    in_=osb[:],
                )

    nc.finalize()
    return nc


_NC = None


def _get_nc():
    global _NC
    if _NC is None:
        _NC = build()
    return _NC


def kernel(x, clusters, clusters2, bn_gamma, bn_beta, _trace=False):
    x = np.ascontiguousarray(np.asarray(x, dtype=np.float32))
    clusters = np.ascontiguousarray(np.asarray(clusters, dtype=np.float32))
    c2t = np.ascontiguousarray(np.asarray(clusters2, dtype=np.float32)[0].T)
    gamma = np.ascontiguousarray(np.asarray(bn_gamma, dtype=np.float32).reshape(K, 1))
    beta = np.ascontiguousarray(np.asarray(bn_beta, dtype=np.float32).reshape(K, 1))

    nc = _get_nc()
    in_maps = [
        {
            "x": np.ascontiguousarray(x[c * BL:(c + 1) * BL]),
            "clusters": clusters,
            "c2t": c2t,
            "gamma": gamma,
            "beta": beta,
        }
        for c in range(N_CORES)
    ]
    res = run_bass_kernel_spmd(
        nc, in_maps, core_ids=list(range(N_CORES)), trace=_trace,
    )
    full = np.concatenate([res.results[c]["vlad"] for c in range(N_CORES)], axis=0)
    out = full.reshape(B, D * K).astype(np.float32)
    if _trace:
        return out, res
    return out
